# revision 25
# baseline (speedup 1.0000x reference)
"""Sliding-window attention (window=256, causal) Trainium2 Bass kernel.

Problem: nn_Attention_31104153157722
  q,k,v: [2, 2048, 1024]; wq/wk/wv/wo: [1024, 1024]; H=16 heads, DH=64.
  out = (softmax(mask(RoPE(q@wqT) @ RoPE(k@wkT)^T / 8)) @ (v@wvT)) @ woT

Sharding: batch(2) x seq-chunk(4) across 8 cores. Each core computes its
512-row output chunk end-to-end, recomputing the 256-row k/v halo locally
(attention is strictly local), so no collectives are needed. Host does
only transposes / fp16 casts / padding / concat.

Device layout is feature-major ("T" layout) throughout:
  qhT/khT [feat, pos], vh [pos, feat], scoresT [key, query]; matmuls are
  fp16 (1 cycle/row on the PE at any moving dim, FWL weight loads) with
  fp32 PSUM accumulation; measured end-to-end rel err ~5e-4.

RoPE: q/k projection weights are row-permuted on the host so each head's
features are [32 even | 32 odd]; RoPE is then a PSUM->SBUF cast (ACT),
2 fp16 multiplies with cos/sin tables, a partition-block swap done by
SBUF->SBUF DMA, and 1 fp16 add (DVE 2x mode).

Softmax: no max-subtraction (scores are O(5)); exp on ScalarE with
scale=1/8 fused, fp16 probs; window mask applied as a 0/1 fp16 multiply;
softmax denominators come free from PV via a ones-column appended to V;
reciprocals are batched [16,256] per super-block on DVE, broadcast via
gpsimd partition_broadcast.
"""

import numpy as np

import concourse.bass as bass
import concourse.mybir as mybir
import concourse.tile as tile
from concourse import bacc, library_config
from concourse.bass_utils import run_bass_kernel_spmd

B, L, DIM = 2, 2048, 1024
H, DH, W = 16, 64, 256
ROPE_BASE = 10000.0
NCORES = 8
CH = L // 4  # 512 rows per core chunk
KL = CH + W  # 768 keys incl halo
NSB = CH // 256  # 2 query super-blocks of 256
NFT = DIM // 128  # 8 feature tiles (2 heads each)
NKT = DIM // 128  # 8 contraction tiles
NKT2 = KL // 128  # 6 key tiles
f32 = mybir.dt.float32
f16 = mybir.dt.float16

_NC_CACHE = [None]


def _rope_emit(nc, p_rope, dst, ps, n, cos_ap, sin_ap, name, ring):
    """dst[f16] = RoPE(ps) with per-head [32 even|32 odd] feature layout.

    dst = xc*cos + swap32(xc*sin_signed); swap32 exchanges adjacent
    32-partition blocks (even<->odd features of each head). The swap is
    4 SBUF->SBUF DMAs on the given HWDGE ring.
    """
    xc = p_rope.tile([128, KL], f16, name=f"xc{name}", tag="xc")
    nc.scalar.copy(xc[:, :n], ps[:, :n])  # PSUM fp32 -> SBUF fp16 on ACT
    m2 = p_rope.tile([128, KL], f16, name=f"m2{name}", tag="m2")
    m2s = p_rope.tile([128, KL], f16, name=f"m2s{name}", tag="m2s")
    nc.vector.tensor_mul(dst[:, :n], xc[:, :n], cos_ap)
    nc.vector.tensor_mul(m2[:, :n], xc[:, :n], sin_ap)
    for b in (0, 64):
        ring.dma_start(m2s[b : b + 32, :n], m2[b + 32 : b + 64, :n])
        ring.dma_start(m2s[b + 32 : b + 64, :n], m2[b : b + 32, :n])
    nc.vector.tensor_add(dst[:, :n], dst[:, :n], m2s[:, :n])


def _build():
    if _NC_CACHE[0] is not None:
        return _NC_CACHE[0]
    nc = bacc.Bacc(None, target_bir_lowering=False, debug=False)

    qT = nc.dram_tensor("qT", [DIM, CH], f16, kind="ExternalInput")
    kT = nc.dram_tensor("kT", [DIM, KL], f16, kind="ExternalInput")
    vT = nc.dram_tensor("vT", [DIM, KL], f16, kind="ExternalInput")
    wqT = nc.dram_tensor("wqT", [DIM, DIM], f16, kind="ExternalInput")
    wkT = nc.dram_tensor("wkT", [DIM, DIM], f16, kind="ExternalInput")
    wvT = nc.dram_tensor("wvT", [DIM, DIM], f16, kind="ExternalInput")
    woT = nc.dram_tensor("woT", [DIM, DIM], f16, kind="ExternalInput")
    cosk = nc.dram_tensor("cosk", [128, KL], f16, kind="ExternalInput")
    sink = nc.dram_tensor("sink", [128, KL], f16, kind="ExternalInput")
    vones = nc.dram_tensor("vones", [128, H], f16, kind="ExternalInput")
    masks = nc.dram_tensor("masks", [128, NSB * 1024], f16, kind="ExternalInput")
    outT = nc.dram_tensor("outT", [DIM, CH], f32, kind="ExternalOutput")

    Exp = mybir.ActivationFunctionType.Exp

    with tile.TileContext(nc) as tc:
        nc.gpsimd.load_library(library_config.attn)
        with (
            tc.tile_pool(name="raw", bufs=24) as p_raw,
            tc.tile_pool(name="w", bufs=24) as p_w,
            tc.tile_pool(name="qh", bufs=NFT) as p_qh,
            tc.tile_pool(name="kh", bufs=NFT) as p_kh,
            tc.tile_pool(name="vh", bufs=NKT2) as p_vh,
            tc.tile_pool(name="att", bufs=NFT) as p_att,
            tc.tile_pool(name="tab", bufs=2) as p_tab,
            tc.tile_pool(name="mask", bufs=1) as p_mask,
            tc.tile_pool(name="rope", bufs=2) as p_rope,
            tc.tile_pool(name="probs", bufs=4) as p_probs,
            tc.tile_pool(name="sv", bufs=3) as p_sv,
            tc.tile_pool(name="small", bufs=2) as p_small,
            tc.tile_pool(name="ob", bufs=2) as p_ob,
            tc.tile_pool(name="big", bufs=3, space="PSUM") as p_big,
            tc.tile_pool(name="ov", bufs=2, space="PSUM") as p_ov,
        ):
            # tables / masks
            t_cos = p_tab.tile([128, KL], f16, name="t_cos", tag="tab")
            t_sin = p_tab.tile([128, KL], f16, name="t_sin", tag="tab")
            nc.scalar.dma_start(t_cos[:], cosk[:])
            nc.scalar.dma_start(t_sin[:], sink[:])
            t_mask = p_mask.tile([128, NSB * 1024], f16, name="t_mask", tag="mask")
            nc.scalar.dma_start(t_mask[:], masks[:])

            # raw activations (feature-major, fp16); tq/twq interleaved so
            # the first projection matmul unblocks as early as possible
            tq, twq = [], []
            for kt in range(NKT):
                t = p_raw.tile([128, CH], f16, name=f"tq{kt}", tag="raw")
                nc.sync.dma_start(t[:], qT[kt * 128 : (kt + 1) * 128, :])
                tq.append(t)
                t = p_w.tile([128, DIM], f16, name=f"twq{kt}", tag="w")
                nc.sync.dma_start(t[:], wqT[kt * 128 : (kt + 1) * 128, :])
                twq.append(t)
            tk = []
            for kt in range(NKT):
                t = p_raw.tile([128, KL], f16, name=f"tk{kt}", tag="raw")
                nc.scalar.dma_start(t[:], kT[kt * 128 : (kt + 1) * 128, :])
                tk.append(t)
            qh = []
            cos_q = t_cos[:, W:KL]
            sin_q = t_sin[:, W:KL]
            for ft in range(NFT):
                ps = p_big.tile([128, 1024], f32, name=f"ps_q{ft}", tag="big")
                for kt in range(NKT):
                    nc.tensor.matmul(
                        ps[:, 0:CH],
                        twq[kt][:, ft * 128 : (ft + 1) * 128],
                        tq[kt][:],
                        start=(kt == 0),
                        stop=(kt == NKT - 1),
                    )
                dst = p_qh.tile([128, CH], f16, name=f"qh{ft}", tag="qh")
                _rope_emit(
                    nc, p_rope, dst, ps, CH, cos_q, sin_q, f"q{ft}",
                    nc.sync if ft % 2 == 0 else nc.scalar,
                )
                qh.append(dst)

            # ---- stage 2: k projection + RoPE -> kh tiles [128, KL] ----
            twk = []
            for kt in range(NKT):
                t = p_w.tile([128, DIM], f16, name=f"twk{kt}", tag="w")
                nc.sync.dma_start(t[:], wkT[kt * 128 : (kt + 1) * 128, :])
                twk.append(t)
            tv = []
            for kt in range(NKT):
                t = p_raw.tile([128, KL], f16, name=f"tv{kt}", tag="raw")
                nc.sync.dma_start(t[:], vT[kt * 128 : (kt + 1) * 128, :])
                tv.append(t)
            twv = []
            for kt in range(NKT):
                t = p_w.tile([128, DIM], f16, name=f"twv{kt}", tag="w")
                nc.sync.dma_start(t[:], wvT[kt * 128 : (kt + 1) * 128, :])
                twv.append(t)
            kh = []
            for ft in range(NFT):
                ps = p_big.tile([128, 1024], f32, name=f"ps_k{ft}", tag="big")
                for kt in range(NKT):
                    nc.tensor.matmul(
                        ps[:, 0:512],
                        twk[kt][:, ft * 128 : (ft + 1) * 128],
                        tk[kt][:, 0:512],
                        start=(kt == 0),
                        stop=(kt == NKT - 1),
                    )
                for kt in range(NKT):
                    nc.tensor.matmul(
                        ps[:, 512:KL],
                        twk[kt][:, ft * 128 : (ft + 1) * 128],
                        tk[kt][:, 512:KL],
                        start=(kt == 0),
                        stop=(kt == NKT - 1),
                    )
                dst = p_kh.tile([128, KL], f16, name=f"kh{ft}", tag="kh")
                _rope_emit(
                    nc, p_rope, dst, ps, KL, t_cos[:], t_sin[:], f"k{ft}",
                    nc.sync if ft % 2 == 0 else nc.scalar,
                )
                kh.append(dst)

            # ---- stage 3: v projection -> vh tiles [128 keys, H*65] ----
            vh = []
            for kt2 in range(NKT2):
                vt = p_vh.tile([128, H * 66], f16, name=f"vh{kt2}", tag="vh")
                for half in range(2):
                    ps = p_big.tile(
                        [128, 1024], f32, name=f"ps_v{kt2}_{half}", tag="big"
                    )
                    for kt in range(NKT):
                        nc.tensor.matmul(
                            ps[:, 0:512],
                            tv[kt][:, kt2 * 128 : (kt2 + 1) * 128],
                            twv[kt][:, half * 512 : (half + 1) * 512],
                            start=(kt == 0),
                            stop=(kt == NKT - 1),
                        )
                    src = ps[:, 0:512].rearrange("p (a b) -> p a b", a=8)
                    dstr = vt[:, half * 8 * 66 : (half + 1) * 8 * 66].rearrange(
                        "p (a b) -> p a b", b=66
                    )[:, :, 0:64]
                    nc.vector.tensor_copy(dstr, src)
                ones_ap = vt.rearrange("p (a b) -> p a b", b=66)[:, :, 64:65]
                nc.scalar.dma_start(
                    ones_ap, vones[:].rearrange("p (a b) -> p a b", b=1)
                )
                vh.append(vt)

            # ---- stage 4: windowed attention ----
            att = [
                p_att.tile([128, CH], f16, name=f"att{ft}", tag="att")
                for ft in range(NFT)
            ]
            two = []
            for kt in range(NKT):
                t = p_w.tile([128, DIM], f16, name=f"two{kt}", tag="w")
                nc.sync.dma_start(t[:], woT[kt * 128 : (kt + 1) * 128, :])
                two.append(t)
            # Attention is software-pipelined one iteration deep: PV and the
            # PSUM->SBUF copy of iteration i-1 are emitted after exp/mask of
            # iteration i, so the ACT queue never head-of-line blocks on PV.
            # Softmax denominators: per 8-head group, one DMA gathers the 8
            # rowsum rows into [8,256], one batched DVE reciprocal, one DMA
            # spreads them back to partition 0, then per-head gpsimd
            # partition_broadcast + normalize-multiply.
            iters = [(sb, j) for sb in range(NSB) for j in range(H // 2)]
            svg = {}  # (sb, g) -> group tile [65, 8*256]
            prs = {}

            def emit_front(idx):
                # two heads (one feature tile) per iteration; their QK
                # matmuls interleave PE row groups 0/64 so each LDWEIGHTS
                # overlaps the other head's in-flight MATMUL
                sb, j = iters[idx]
                ft = j
                scs = []
                for hs in range(2):
                    scs.append(
                        p_big.tile(
                            [128, 1024], f32, name=f"sc{sb}_{j}_{hs}", tag="big"
                        )
                    )
                for t in range(4):
                    kt2 = 2 * sb + t
                    for hs in range(2):
                        poff = hs * 64
                        nc.tensor.matmul(
                            scs[hs][:, t * 256 : (t + 1) * 256],
                            kh[ft][poff : poff + 64, kt2 * 128 : (kt2 + 1) * 128],
                            qh[ft][poff : poff + 64, sb * 256 : (sb + 1) * 256],
                            start=True,
                            stop=True,
                        )
                pair = []
                for hs in range(2):
                    h = 2 * j + hs
                    pr = p_probs.tile(
                        [128, 1024], f16, name=f"pr{sb}_{h}", tag="pr"
                    )
                    nc.scalar.activation(
                        pr[:], scs[hs][:], Exp, scale=float(DH) ** -0.5
                    )
                    eng = nc.vector if hs == 0 else nc.gpsimd
                    eng.tensor_mul(
                        pr[:], pr[:], t_mask[:, sb * 1024 : (sb + 1) * 1024]
                    )
                    pair.append(pr)
                prs[idx] = pair

            def emit_back(idx):
                sb, j = iters[idx]
                pair = prs.pop(idx)
                for hs in range(2):
                    h = 2 * j + hs
                    g, hh = h // 8, h % 8
                    pr = pair[hs]
                    ov = p_ov.tile([128, 256], f32, name=f"ov{sb}_{h}", tag="ov")
                    for t in range(4):
                        kt2 = 2 * sb + t
                        nc.tensor.matmul(
                            ov[0:65, :],
                            vh[kt2][:, h * 66 : h * 66 + 65],
                            pr[:, t * 256 : (t + 1) * 256],
                            start=(t == 0),
                            stop=(t == 3),
                        )
                    if hh == 0:
                        svg[(sb, g)] = p_sv.tile(
                            [65, 8 * 256], f32, name=f"svg{sb}_{g}", tag="sv"
                        )
                    nc.scalar.copy(
                        svg[(sb, g)][:, hh * 256 : (hh + 1) * 256], ov[0:65, :]
                    )
                    if hh == 7:
                        emit_norm(sb, g)

            def emit_norm(sb, g):
                sg = svg[(sb, g)]
                rsum = p_small.tile([8, 256], f32, name=f"rs{sb}_{g}", tag="rsum")
                nc.sync.dma_start(rsum[:], sg[64:65, :])
                rrec = p_small.tile([8, 256], f32, name=f"rr{sb}_{g}", tag="rrec")
                nc.vector.reciprocal_approx_fast(rrec[:], rsum[:])
                rtw = p_small.tile([1, 8 * 256], f32, name=f"rtw{sb}_{g}", tag="rt")
                nc.sync.dma_start(rtw[:], rrec[:])
                for hh in range(8):
                    h = g * 8 + hh
                    ft, poff = h // 2, (h % 2) * 64
                    rb = p_small.tile([64, 256], f32, name=f"rb{sb}_{h}", tag="rb")
                    nc.gpsimd.partition_broadcast(
                        rb[:], rtw[0:1, hh * 256 : (hh + 1) * 256]
                    )
                    nc.vector.tensor_mul(
                        att[ft][poff : poff + 64, sb * 256 : (sb + 1) * 256],
                        sg[0:64, hh * 256 : (hh + 1) * 256],
                        rb[:],
                    )

            for idx in range(len(iters) + 1):
                if idx < len(iters):
                    emit_front(idx)
                if idx >= 1:
                    emit_back(idx - 1)

            # ---- stage 5: output projection ----
            for ot in range(NFT):
                ps = p_big.tile([128, 1024], f32, name=f"ps_o{ot}", tag="big")
                for ftk in range(NKT):
                    nc.tensor.matmul(
                        ps[:, 0:CH],
                        two[ftk][:, ot * 128 : (ot + 1) * 128],
                        att[ftk][:],
                        start=(ftk == 0),
                        stop=(ftk == NKT - 1),
                    )
                ob = p_ob.tile([128, CH], f32, name=f"ob{ot}", tag="ob")
                nc.scalar.copy(ob[:], ps[:, 0:CH])
                nc.sync.dma_start(outT[ot * 128 : (ot + 1) * 128, :], ob[:])

    nc.compile()
    _NC_CACHE[0] = nc
    return nc


def make_in_maps(q, k, v, wq, wk, wv, wo):
    q = np.asarray(q, np.float32)
    k = np.asarray(k, np.float32)
    v = np.asarray(v, np.float32)
    wq = np.asarray(wq, np.float32)
    wk = np.asarray(wk, np.float32)
    wv = np.asarray(wv, np.float32)
    wo = np.asarray(wo, np.float32)

    # per-head feature permutation: [even, odd] so RoPE pairs sit 32 apart
    hp = np.concatenate([np.arange(0, DH, 2), np.arange(1, DH, 2)])
    perm = (np.arange(H)[:, None] * DH + hp[None, :]).reshape(-1)
    wqT = np.ascontiguousarray(wq[perm].T.astype(np.float16))
    wkT = np.ascontiguousarray(wk[perm].T.astype(np.float16))
    wvT = np.ascontiguousarray(wv.T.astype(np.float16))
    woT = np.ascontiguousarray(wo.T.astype(np.float16))

    freqs = 1.0 / (ROPE_BASE ** (np.arange(0, DH, 2, dtype=np.float64) / DH))
    m_of_p = np.tile(np.arange(32), 4)
    sign = np.where((np.arange(128) % 64) < 32, 1.0, -1.0)[:, None]

    in_maps = []
    for cid in range(NCORES):
        b, c = divmod(cid, 4)
        s = c * CH
        lo = s - W
        qTc = np.ascontiguousarray(q[b, s : s + CH].T.astype(np.float16))
        kpad = np.zeros((KL, DIM), np.float32)
        vpad = np.zeros((KL, DIM), np.float32)
        src_lo = max(lo, 0)
        kpad[src_lo - lo :] = k[b, src_lo : s + CH]
        vpad[src_lo - lo :] = v[b, src_lo : s + CH]
        kTc = np.ascontiguousarray(kpad.T.astype(np.float16))
        vTc = np.ascontiguousarray(vpad.T.astype(np.float16))

        pos_k = (lo + np.arange(KL)).astype(np.float64)
        ang = freqs[m_of_p][:, None] * pos_k[None, :]
        cosk = np.cos(ang).astype(np.float16)
        sink = (np.sin(ang) * sign).astype(np.float16)

        masks = np.zeros((128, NSB * 1024), np.float16)
        j = np.arange(128)[:, None]
        i = np.arange(256)[None, :]
        for sb in range(NSB):
            for t in range(4):
                kt2 = 2 * sb + t
                K = lo + kt2 * 128 + j
                Q = s + sb * 256 + i
                valid = (K >= 0) & (K <= Q) & (K >= Q - W)
                masks[:, sb * 1024 + t * 256 : sb * 1024 + (t + 1) * 256] = valid
        in_maps.append(
            dict(
                qT=qTc,
                kT=kTc,
                vT=vTc,
                wqT=wqT,
                wkT=wkT,
                wvT=wvT,
                woT=woT,
                cosk=cosk,
                sink=sink,
                vones=np.ones((128, H), np.float16),
                masks=masks,
            )
        )
    return in_maps


def gather(results):
    out = np.empty((B, L, DIM), np.float32)
    for cid in range(NCORES):
        b, c = divmod(cid, 4)
        out[b, c * CH : (c + 1) * CH, :] = results[cid]["outT"].T
    return out


def kernel(**inputs):
    nc = _build()
    in_maps = make_in_maps(**inputs)
    res = run_bass_kernel_spmd(nc, in_maps, core_ids=list(range(NCORES)))
    return gather(res.results)


# revision 27
# speedup vs baseline: 1.5632x; 1.5632x over previous
"""Sliding-window attention (window=256, causal) Trainium2 Bass kernel.

Problem: nn_Attention_31104153157722
  q,k,v: [2, 2048, 1024]; wq/wk/wv/wo: [1024, 1024]; H=16 heads, DH=64.
  out = (softmax(mask(RoPE(q@wqT) @ RoPE(k@wkT)^T / 8)) @ (v@wvT)) @ woT

Sharding: batch(2) x seq-chunk(4) across 8 cores. Each core computes its
512-row output chunk end-to-end, recomputing the 256-row k/v halo locally
(attention is strictly local), so no collectives are needed. Host does
only transposes / fp16 casts / padding / concat.

Device layout is feature-major ("T" layout) throughout:
  qhT/khT [feat, pos], vh [pos, feat], scoresT [key, query]; matmuls are
  fp16 (1 cycle/row on the PE at any moving dim, FWL weight loads) with
  fp32 PSUM accumulation; measured end-to-end rel err ~5e-4.

RoPE: q/k projection weights are row-permuted on the host so each head's
features are [32 even | 32 odd]; RoPE is then a PSUM->SBUF cast (ACT),
2 fp16 multiplies with cos/sin tables, a partition-block swap done by
SBUF->SBUF DMA, and 1 fp16 add (DVE 2x mode).

Softmax: no max-subtraction (scores are O(5)); exp on ScalarE with
scale=1/8 fused, fp16 probs; window mask applied as a 0/1 fp16 multiply;
softmax denominators come free from PV via a ones-column appended to V;
reciprocals are batched [16,256] per super-block on DVE, broadcast via
gpsimd partition_broadcast.
"""

import numpy as np

import concourse.bass as bass
import concourse.mybir as mybir
import concourse.tile as tile
from concourse import bacc, library_config
from concourse.bass_utils import run_bass_kernel_spmd

B, L, DIM = 2, 2048, 1024
H, DH, W = 16, 64, 256
ROPE_BASE = 10000.0
NCORES = 8
CH = L // 4  # 512 rows per core chunk
KL = CH + W  # 768 keys incl halo
NSB = CH // 256  # 2 query super-blocks of 256
NFT = DIM // 128  # 8 feature tiles (2 heads each)
NKT = DIM // 128  # 8 contraction tiles
NKT2 = KL // 128  # 6 key tiles
f32 = mybir.dt.float32
f16 = mybir.dt.float16

_NC_CACHE = [None]


def _rope_emit(nc, p_rope, dst, ps, n, cos_ap, sin_ap, name, ring):
    """dst[f16] = RoPE(ps) with per-head [32 even|32 odd] feature layout.

    dst = xc*cos + swap32(xc*sin_signed); swap32 exchanges adjacent
    32-partition blocks (even<->odd features of each head). The swap is
    4 SBUF->SBUF DMAs on the given HWDGE ring.
    """
    xc = p_rope.tile([128, KL], f16, name=f"xc{name}", tag="xc")
    nc.vector.tensor_copy(xc[:, :n], ps[:, :n])  # PSUM fp32 -> SBUF fp16
    m2 = p_rope.tile([128, KL], f16, name=f"m2{name}", tag="m2")
    m2s = p_rope.tile([128, KL], f16, name=f"m2s{name}", tag="m2s")
    nc.vector.tensor_mul(dst[:, :n], xc[:, :n], cos_ap)
    nc.vector.tensor_mul(m2[:, :n], xc[:, :n], sin_ap)
    for b in (0, 64):
        ring.dma_start(m2s[b : b + 32, :n], m2[b + 32 : b + 64, :n])
        ring.dma_start(m2s[b + 32 : b + 64, :n], m2[b : b + 32, :n])
    nc.vector.tensor_add(dst[:, :n], dst[:, :n], m2s[:, :n])


def _build():
    if _NC_CACHE[0] is not None:
        return _NC_CACHE[0]
    nc = bacc.Bacc(None, target_bir_lowering=False, debug=False)

    qT = nc.dram_tensor("qT", [DIM, CH], f16, kind="ExternalInput")
    kT = nc.dram_tensor("kT", [DIM, KL], f16, kind="ExternalInput")
    vT = nc.dram_tensor("vT", [DIM, KL], f16, kind="ExternalInput")
    wqT = nc.dram_tensor("wqT", [DIM, DIM], f16, kind="ExternalInput")
    wkT = nc.dram_tensor("wkT", [DIM, DIM], f16, kind="ExternalInput")
    wvT = nc.dram_tensor("wvT", [DIM, DIM], f16, kind="ExternalInput")
    woT = nc.dram_tensor("woT", [DIM, DIM], f16, kind="ExternalInput")
    cosk = nc.dram_tensor("cosk", [128, KL], f16, kind="ExternalInput")
    sink = nc.dram_tensor("sink", [128, KL], f16, kind="ExternalInput")
    vones = nc.dram_tensor("vones", [128, H], f16, kind="ExternalInput")
    masks = nc.dram_tensor("masks", [128, NSB * 1024], f16, kind="ExternalInput")
    outT = nc.dram_tensor("outT", [DIM, CH], f32, kind="ExternalOutput")

    Exp = mybir.ActivationFunctionType.Exp

    with tile.TileContext(nc) as tc:
        nc.gpsimd.load_library(library_config.attn)
        with (
            tc.tile_pool(name="raw", bufs=24) as p_raw,
            tc.tile_pool(name="w", bufs=24) as p_w,
            tc.tile_pool(name="qh", bufs=NFT) as p_qh,
            tc.tile_pool(name="kh", bufs=NFT) as p_kh,
            tc.tile_pool(name="vh", bufs=NKT2) as p_vh,
            tc.tile_pool(name="att", bufs=NFT) as p_att,
            tc.tile_pool(name="tab", bufs=2) as p_tab,
            tc.tile_pool(name="mask", bufs=1) as p_mask,
            tc.tile_pool(name="rope", bufs=2) as p_rope,
            tc.tile_pool(name="probs", bufs=4) as p_probs,
            tc.tile_pool(name="sv", bufs=3) as p_sv,
            tc.tile_pool(name="small", bufs=2) as p_small,
            tc.tile_pool(name="ob", bufs=2) as p_ob,
            tc.tile_pool(name="big", bufs=3, space="PSUM") as p_big,
            tc.tile_pool(name="ov", bufs=2, space="PSUM") as p_ov,
        ):
            # tables / masks
            t_cos = p_tab.tile([128, KL], f16, name="t_cos", tag="tab")
            t_sin = p_tab.tile([128, KL], f16, name="t_sin", tag="tab")
            nc.scalar.dma_start(t_cos[:], cosk[:])
            nc.scalar.dma_start(t_sin[:], sink[:])
            t_mask = p_mask.tile([128, NSB * 1024], f16, name="t_mask", tag="mask")
            nc.scalar.dma_start(t_mask[:], masks[:])

            # raw activations (feature-major, fp16); tq/twq interleaved so
            # the first projection matmul unblocks as early as possible
            tq, twq = [], []
            for kt in range(NKT):
                t = p_raw.tile([128, CH], f16, name=f"tq{kt}", tag="raw")
                nc.sync.dma_start(t[:], qT[kt * 128 : (kt + 1) * 128, :])
                tq.append(t)
                t = p_w.tile([128, DIM], f16, name=f"twq{kt}", tag="w")
                nc.sync.dma_start(t[:], wqT[kt * 128 : (kt + 1) * 128, :])
                twq.append(t)
            tk = []
            for kt in range(NKT):
                t = p_raw.tile([128, KL], f16, name=f"tk{kt}", tag="raw")
                nc.scalar.dma_start(t[:], kT[kt * 128 : (kt + 1) * 128, :])
                tk.append(t)
            qh = []
            cos_q = t_cos[:, W:KL]
            sin_q = t_sin[:, W:KL]
            for ft in range(NFT):
                ps = p_big.tile([128, 1024], f32, name=f"ps_q{ft}", tag="big")
                for kt in range(NKT):
                    nc.tensor.matmul(
                        ps[:, 0:CH],
                        twq[kt][:, ft * 128 : (ft + 1) * 128],
                        tq[kt][:],
                        start=(kt == 0),
                        stop=(kt == NKT - 1),
                    )
                dst = p_qh.tile([128, CH], f16, name=f"qh{ft}", tag="qh")
                _rope_emit(
                    nc, p_rope, dst, ps, CH, cos_q, sin_q, f"q{ft}",
                    nc.sync if ft % 2 == 0 else nc.scalar,
                )
                qh.append(dst)

            # ---- stage 2: k projection + RoPE -> kh tiles [128, KL] ----
            twk = []
            for kt in range(NKT):
                t = p_w.tile([128, DIM], f16, name=f"twk{kt}", tag="w")
                nc.sync.dma_start(t[:], wkT[kt * 128 : (kt + 1) * 128, :])
                twk.append(t)
            tv = []
            for kt in range(NKT):
                t = p_raw.tile([128, KL], f16, name=f"tv{kt}", tag="raw")
                nc.sync.dma_start(t[:], vT[kt * 128 : (kt + 1) * 128, :])
                tv.append(t)
            twv = []
            for kt in range(NKT):
                t = p_w.tile([128, DIM], f16, name=f"twv{kt}", tag="w")
                nc.sync.dma_start(t[:], wvT[kt * 128 : (kt + 1) * 128, :])
                twv.append(t)
            kh = []
            for ft in range(NFT):
                ps = p_big.tile([128, 1024], f32, name=f"ps_k{ft}", tag="big")
                for kt in range(NKT):
                    nc.tensor.matmul(
                        ps[:, 0:512],
                        twk[kt][:, ft * 128 : (ft + 1) * 128],
                        tk[kt][:, 0:512],
                        start=(kt == 0),
                        stop=(kt == NKT - 1),
                    )
                for kt in range(NKT):
                    nc.tensor.matmul(
                        ps[:, 512:KL],
                        twk[kt][:, ft * 128 : (ft + 1) * 128],
                        tk[kt][:, 512:KL],
                        start=(kt == 0),
                        stop=(kt == NKT - 1),
                    )
                dst = p_kh.tile([128, KL], f16, name=f"kh{ft}", tag="kh")
                _rope_emit(
                    nc, p_rope, dst, ps, KL, t_cos[:], t_sin[:], f"k{ft}",
                    nc.sync if ft % 2 == 0 else nc.scalar,
                )
                kh.append(dst)

            # ---- stage 3: v projection -> vh tiles [128 keys, H*65] ----
            vh = []
            for kt2 in range(NKT2):
                vt = p_vh.tile([128, H * 66], f16, name=f"vh{kt2}", tag="vh")
                for half in range(2):
                    ps = p_big.tile(
                        [128, 1024], f32, name=f"ps_v{kt2}_{half}", tag="big"
                    )
                    for kt in range(NKT):
                        nc.tensor.matmul(
                            ps[:, 0:512],
                            tv[kt][:, kt2 * 128 : (kt2 + 1) * 128],
                            twv[kt][:, half * 512 : (half + 1) * 512],
                            start=(kt == 0),
                            stop=(kt == NKT - 1),
                        )
                    src = ps[:, 0:512].rearrange("p (a b) -> p a b", a=8)
                    dstr = vt[:, half * 8 * 66 : (half + 1) * 8 * 66].rearrange(
                        "p (a b) -> p a b", b=66
                    )[:, :, 0:64]
                    nc.vector.tensor_copy(dstr, src)
                ones_ap = vt.rearrange("p (a b) -> p a b", b=66)[:, :, 64:65]
                nc.scalar.dma_start(
                    ones_ap, vones[:].rearrange("p (a b) -> p a b", b=1)
                )
                vh.append(vt)

            # ---- stage 4: windowed attention ----
            att = [
                p_att.tile([128, CH], f16, name=f"att{ft}", tag="att")
                for ft in range(NFT)
            ]
            two = []
            for kt in range(NKT):
                t = p_w.tile([128, DIM], f16, name=f"two{kt}", tag="w")
                nc.sync.dma_start(t[:], woT[kt * 128 : (kt + 1) * 128, :])
                two.append(t)
            # Attention is software-pipelined one iteration deep: PV and the
            # PSUM->SBUF copy of iteration i-1 are emitted after exp/mask of
            # iteration i, so the ACT queue never head-of-line blocks on PV.
            # Softmax denominators: per 8-head group, one DMA gathers the 8
            # rowsum rows into [8,256], one batched DVE reciprocal, one DMA
            # spreads them back to partition 0, then per-head gpsimd
            # partition_broadcast + normalize-multiply.
            iters = [(sb, j) for sb in range(NSB) for j in range(H // 2)]
            svg = {}  # (sb, g) -> group tile [65, 8*256]
            prs = {}

            def emit_front(idx):
                # two heads (one feature tile) per iteration; their QK
                # matmuls interleave PE row groups 0/64 so each LDWEIGHTS
                # overlaps the other head's in-flight MATMUL
                sb, j = iters[idx]
                ft = j
                scs = []
                for hs in range(2):
                    scs.append(
                        p_big.tile(
                            [128, 1024], f32, name=f"sc{sb}_{j}_{hs}", tag="big"
                        )
                    )
                for t in range(4):
                    kt2 = 2 * sb + t
                    for hs in range(2):
                        poff = hs * 64
                        nc.tensor.matmul(
                            scs[hs][:, t * 256 : (t + 1) * 256],
                            kh[ft][poff : poff + 64, kt2 * 128 : (kt2 + 1) * 128],
                            qh[ft][poff : poff + 64, sb * 256 : (sb + 1) * 256],
                            start=True,
                            stop=True,
                        )
                pair = []
                for hs in range(2):
                    h = 2 * j + hs
                    pr = p_probs.tile(
                        [128, 1024], f16, name=f"pr{sb}_{h}", tag="pr"
                    )
                    nc.scalar.activation(
                        pr[:], scs[hs][:], Exp, scale=float(DH) ** -0.5
                    )
                    nc.vector.tensor_mul(
                        pr[:], pr[:], t_mask[:, sb * 1024 : (sb + 1) * 1024]
                    )
                    pair.append(pr)
                prs[idx] = pair

            def emit_back(idx):
                sb, j = iters[idx]
                pair = prs.pop(idx)
                for hs in range(2):
                    h = 2 * j + hs
                    g, hh = h // 8, h % 8
                    pr = pair[hs]
                    ov = p_ov.tile([128, 256], f32, name=f"ov{sb}_{h}", tag="ov")
                    for t in range(4):
                        kt2 = 2 * sb + t
                        nc.tensor.matmul(
                            ov[0:65, :],
                            vh[kt2][:, h * 66 : h * 66 + 65],
                            pr[:, t * 256 : (t + 1) * 256],
                            start=(t == 0),
                            stop=(t == 3),
                        )
                    if hh == 0:
                        svg[(sb, g)] = p_sv.tile(
                            [65, 8 * 256], f32, name=f"svg{sb}_{g}", tag="sv"
                        )
                    nc.scalar.copy(
                        svg[(sb, g)][:, hh * 256 : (hh + 1) * 256], ov[0:65, :]
                    )
                    if hh == 7:
                        emit_norm(sb, g)

            def emit_norm(sb, g):
                sg = svg[(sb, g)]
                rsum = p_small.tile([8, 256], f32, name=f"rs{sb}_{g}", tag="rsum")
                nc.sync.dma_start(rsum[:], sg[64:65, :])
                rrec = p_small.tile([8, 256], f32, name=f"rr{sb}_{g}", tag="rrec")
                nc.vector.reciprocal_approx_fast(rrec[:], rsum[:])
                rtw = p_small.tile([1, 8 * 256], f32, name=f"rtw{sb}_{g}", tag="rt")
                nc.sync.dma_start(rtw[:], rrec[:])
                for hh in range(8):
                    h = g * 8 + hh
                    ft, poff = h // 2, (h % 2) * 64
                    rb = p_small.tile([64, 256], f32, name=f"rb{sb}_{h}", tag="rb")
                    nc.gpsimd.partition_broadcast(
                        rb[:], rtw[0:1, hh * 256 : (hh + 1) * 256]
                    )
                    nc.vector.tensor_mul(
                        att[ft][poff : poff + 64, sb * 256 : (sb + 1) * 256],
                        sg[0:64, hh * 256 : (hh + 1) * 256],
                        rb[:],
                    )

            for idx in range(len(iters) + 1):
                if idx < len(iters):
                    emit_front(idx)
                if idx >= 1:
                    emit_back(idx - 1)

            # ---- stage 5: output projection ----
            for ot in range(NFT):
                ps = p_big.tile([128, 1024], f32, name=f"ps_o{ot}", tag="big")
                for ftk in range(NKT):
                    nc.tensor.matmul(
                        ps[:, 0:CH],
                        two[ftk][:, ot * 128 : (ot + 1) * 128],
                        att[ftk][:],
                        start=(ftk == 0),
                        stop=(ftk == NKT - 1),
                    )
                ob = p_ob.tile([128, CH], f32, name=f"ob{ot}", tag="ob")
                nc.scalar.copy(ob[:], ps[:, 0:CH])
                nc.sync.dma_start(outT[ot * 128 : (ot + 1) * 128, :], ob[:])

    nc.compile()
    _NC_CACHE[0] = nc
    return nc


def make_in_maps(q, k, v, wq, wk, wv, wo):
    q = np.asarray(q, np.float32)
    k = np.asarray(k, np.float32)
    v = np.asarray(v, np.float32)
    wq = np.asarray(wq, np.float32)
    wk = np.asarray(wk, np.float32)
    wv = np.asarray(wv, np.float32)
    wo = np.asarray(wo, np.float32)

    # per-head feature permutation: [even, odd] so RoPE pairs sit 32 apart
    hp = np.concatenate([np.arange(0, DH, 2), np.arange(1, DH, 2)])
    perm = (np.arange(H)[:, None] * DH + hp[None, :]).reshape(-1)
    wqT = np.ascontiguousarray(wq[perm].T.astype(np.float16))
    wkT = np.ascontiguousarray(wk[perm].T.astype(np.float16))
    wvT = np.ascontiguousarray(wv.T.astype(np.float16))
    woT = np.ascontiguousarray(wo.T.astype(np.float16))

    freqs = 1.0 / (ROPE_BASE ** (np.arange(0, DH, 2, dtype=np.float64) / DH))
    m_of_p = np.tile(np.arange(32), 4)
    sign = np.where((np.arange(128) % 64) < 32, 1.0, -1.0)[:, None]

    in_maps = []
    for cid in range(NCORES):
        b, c = divmod(cid, 4)
        s = c * CH
        lo = s - W
        qTc = np.ascontiguousarray(q[b, s : s + CH].T.astype(np.float16))
        kpad = np.zeros((KL, DIM), np.float32)
        vpad = np.zeros((KL, DIM), np.float32)
        src_lo = max(lo, 0)
        kpad[src_lo - lo :] = k[b, src_lo : s + CH]
        vpad[src_lo - lo :] = v[b, src_lo : s + CH]
        kTc = np.ascontiguousarray(kpad.T.astype(np.float16))
        vTc = np.ascontiguousarray(vpad.T.astype(np.float16))

        pos_k = (lo + np.arange(KL)).astype(np.float64)
        ang = freqs[m_of_p][:, None] * pos_k[None, :]
        cosk = np.cos(ang).astype(np.float16)
        sink = (np.sin(ang) * sign).astype(np.float16)

        masks = np.zeros((128, NSB * 1024), np.float16)
        j = np.arange(128)[:, None]
        i = np.arange(256)[None, :]
        for sb in range(NSB):
            for t in range(4):
                kt2 = 2 * sb + t
                K = lo + kt2 * 128 + j
                Q = s + sb * 256 + i
                valid = (K >= 0) & (K <= Q) & (K >= Q - W)
                masks[:, sb * 1024 + t * 256 : sb * 1024 + (t + 1) * 256] = valid
        in_maps.append(
            dict(
                qT=qTc,
                kT=kTc,
                vT=vTc,
                wqT=wqT,
                wkT=wkT,
                wvT=wvT,
                woT=woT,
                cosk=cosk,
                sink=sink,
                vones=np.ones((128, H), np.float16),
                masks=masks,
            )
        )
    return in_maps


def gather(results):
    out = np.empty((B, L, DIM), np.float32)
    for cid in range(NCORES):
        b, c = divmod(cid, 4)
        out[b, c * CH : (c + 1) * CH, :] = results[cid]["outT"].T
    return out


def kernel(**inputs):
    nc = _build()
    in_maps = make_in_maps(**inputs)
    res = run_bass_kernel_spmd(nc, in_maps, core_ids=list(range(NCORES)))
    return gather(res.results)
